# revision 1
# baseline (speedup 1.0000x reference)
"""HNN layer kernel — nn_HNNLayer_59124519796864 (Bass/Tile, 8 NeuronCores).

Contract: kernel(**inputs) takes FULL unsharded inputs
  h [262144, 256] f32, weight [256, 256] f32, bias [1, 256] f32,
  gamma [256] f32, beta [256] f32
and returns the FULL [262144, 256] f32 output.

Sharding: pure data parallel — rows split into 8 contiguous shards, one per
core; weight/bias replicated. No collectives.

Math (c_in = c_out = 1). The reference chain
  logmap0 -> LayerNorm -> expmap0 -> logmap0 -> W -> expmap0
  -> (transp0 + expmap bias add) -> relu-in-tangent -> expmap0
collapses substantially:
  * LayerNorm is invariant to the per-row positive scale of logmap0(h), so
    x-side work reduces to row stats of h plus one fused scale op:
        xn = (h - mu_h) * g,  g = f_h * rsqrt(f_h^2*var_h + eps),
        f_h = atanh(min(|h|,B))/|h|
  * logmap0(expmap0(xn)) = min(|xn|, atanh(B)) * xn/|xn| = s*xn, and s
    commutes with the GEMM: xt = s*(xn @ W.T).
  * The hyperbolic bias add collapses: lam*|b|/2 == |bias| identically, so
    `second` = tanh(|bias|)*bias/|bias| = v is a constant vector; the mobius
    add becomes out_ma = pa*y + q*v with per-row scalars pa, q computed from
    |y|^2 and y.v (y.v rides the GEMM as a 257th output column W^T v).
  * HypAct folds to out = c34 * relu(t2'), t2' = y + (q/pa)*v.
All transcendentals are built from Ln/Exp only (one ACT table set, no
table-switch thrash): rsqrt(x)=exp(-.5 ln x), 1/x=exp(-ln x),
tanh(x)=1-2/(exp(2x)+1).
"""
import sys

sys.path.insert(0, "/opt/trn_rl_repo")

import numpy as np
import ml_dtypes
from contextlib import ExitStack

N_FULL, D = 262144, 256
N_CORES = 8
P = 128
RPC = N_FULL // N_CORES          # rows per core
TILES = RPC // P                 # 256 tiles of 128 rows
TG = 32                          # tiles per scalar group
GROUPS = TILES // TG
OG_CH = 8                        # tiles per output store chunk

BOUND = 1.0 - 1e-5
ATB = float(np.arctanh(np.float64(BOUND)))   # atanh clip plateau
LN_EPS = 1e-5
TINY = 1e-30

_bf16 = ml_dtypes.bfloat16


def _build_nc(v2: float):
    from concourse import bass, bacc, tile, masks, mybir

    f32 = mybir.dt.float32
    bf16 = mybir.dt.bfloat16
    FT = mybir.ActivationFunctionType
    OP = mybir.AluOpType

    nc = bacc.Bacc(
        "TRN2",
        target_bir_lowering=False,
        debug=False,
        num_devices=N_CORES,
    )
    h_ext = nc.declare_dram_parameter("h", [RPC, D], f32, isOutput=False)
    rhs_ext = nc.declare_dram_parameter("rhs", [P, 2, D + 1], bf16, isOutput=False)
    vb_ext = nc.declare_dram_parameter("vb", [P, D], bf16, isOutput=False)
    out_ext = nc.declare_dram_parameter("out", [RPC, D], f32, isOutput=True)

    with tile.TileContext(nc) as tc, ExitStack() as ctx:
        V, S, G_, PE = nc.vector, nc.scalar, nc.gpsimd, nc.tensor

        const = ctx.enter_context(tc.tile_pool(name="const", bufs=1))
        ident = const.tile([P, P], bf16)
        masks.make_identity(nc, ident[:])
        rhs_sb = const.tile([P, 2, D + 1], bf16)
        nc.sync.dma_start(rhs_sb[:], rhs_ext[:])
        vb_sb = const.tile([P, D], bf16)
        nc.sync.dma_start(vb_sb[:], vb_ext[:])

        hp = ctx.enter_context(tc.tile_pool(name="hp", bufs=2))
        stp = ctx.enter_context(tc.tile_pool(name="stp", bufs=2))
        sc = ctx.enter_context(tc.tile_pool(name="sc", bufs=2))
        xnp = ctx.enter_context(tc.tile_pool(name="xnp", bufs=4))
        xtp = ctx.enter_context(tc.tile_pool(name="xtp", bufs=4))
        yp = ctx.enter_context(tc.tile_pool(name="yp", bufs=2))
        t2p = ctx.enter_context(tc.tile_pool(name="t2p", bufs=2))
        scrp = ctx.enter_context(tc.tile_pool(name="scrp", bufs=4))
        ogp = ctx.enter_context(tc.tile_pool(name="ogp", bufs=2))
        pst = ctx.enter_context(tc.tile_pool(name="pst", bufs=3, space="PSUM"))
        psy = ctx.enter_context(tc.tile_pool(name="psy", bufs=3, space="PSUM"))

        def sct(tag):
            return sc.tile([P, TG], f32, tag=tag, name=tag)

        for g in range(GROUPS):
            hslab = h_ext[bass.ts(g, TG * P), :].rearrange("(t p) d -> p t d", p=P)
            hg = hp.tile([P, TG, D], f32, tag="hg", name="hg")
            nc.sync.dma_start(hg[:], hslab)

            # --- per-tile row stats of h ---------------------------------
            stg = stp.tile([P, TG, 6], f32, tag="stg", name="stg")
            for t in range(TG):
                V.bn_stats(stg[:, t, :], hg[:, t, :])

            # --- phase A scalars (gpsimd + ACT): mu, g, s ----------------
            me, M2e = stg[:, :, 1], stg[:, :, 2]
            mo, M2o = stg[:, :, 4], stg[:, :, 5]
            d = sct("d");      V.tensor_sub(d[:], me, mo)
            mu2 = sct("mu2");  V.tensor_add(mu2[:], me, mo)
            mu = sct("mu");    V.tensor_scalar_mul(mu[:], mu2[:], 0.5)
            m2eo = sct("m2eo"); V.tensor_add(m2eo[:], M2e, M2o)
            dd = sct("dd");    V.tensor_mul(dd[:], d[:], d[:])
            m2 = sct("m2");    V.scalar_tensor_tensor(m2[:], dd[:], 64.0, m2eo[:], OP.mult, OP.add)
            muq = sct("muq");  V.tensor_mul(muq[:], mu[:], mu[:])
            nh2 = sct("nh2");  V.scalar_tensor_tensor(nh2[:], muq[:], 256.0, m2[:], OP.mult, OP.add)
            nh2c = sct("nh2c"); V.tensor_scalar_max(nh2c[:], nh2[:], TINY)
            lnh = sct("lnh");  S.activation(lnh[:], nh2c[:], FT.Ln)
            nh = sct("nh");    S.activation(nh[:], lnh[:], FT.Exp, scale=0.5)
            rnh = sct("rnh");  S.activation(rnh[:], lnh[:], FT.Exp, scale=-0.5)
            z = sct("z");      V.tensor_scalar_min(z[:], nh[:], BOUND)
            zn = sct("zn");    V.tensor_scalar_add(zn[:], z[:], 1.0)
            zd = sct("zd");    V.tensor_scalar(zd[:], z[:], -1.0, 1.0, OP.mult, OP.add)
            l1 = sct("l1");    S.activation(l1[:], zn[:], FT.Ln)
            l2 = sct("l2");    S.activation(l2[:], zd[:], FT.Ln)
            dl = sct("dl");    V.tensor_sub(dl[:], l1[:], l2[:])
            fh = sct("fh");    V.scalar_tensor_tensor(fh[:], dl[:], 0.5, rnh[:], OP.mult, OP.mult)
            fh2 = sct("fh2");  V.tensor_mul(fh2[:], fh[:], fh[:])
            u = sct("u");      V.scalar_tensor_tensor(u[:], fh2[:], 1.0 / 256.0, m2[:], OP.mult, OP.mult)
            u2 = sct("u2");    V.tensor_scalar_add(u2[:], u[:], LN_EPS)
            lu2 = sct("lu2");  S.activation(lu2[:], u2[:], FT.Ln)
            gs = sct("gs");    S.activation(gs[:], lu2[:], FT.Exp, scale=-0.5)
            gsc = sct("gsc");  V.tensor_mul(gsc[:], fh[:], gs[:])
            gg = sct("gg");    V.tensor_mul(gg[:], gsc[:], gsc[:])
            nn2 = sct("nn2");  V.tensor_mul(nn2[:], gg[:], m2[:])
            nn2c = sct("nn2c"); V.tensor_scalar_max(nn2c[:], nn2[:], TINY)
            lnn = sct("lnn");  S.activation(lnn[:], nn2c[:], FT.Ln)
            nn = sct("nn");    S.activation(nn[:], lnn[:], FT.Exp, scale=0.5)
            rnn = sct("rnn");  S.activation(rnn[:], lnn[:], FT.Exp, scale=-0.5)
            nmn = sct("nmn");  V.tensor_scalar_min(nmn[:], nn[:], ATB)
            s_ = sct("s_");    V.tensor_mul(s_[:], nmn[:], rnn[:])

            # --- per-tile: xn, transpose, GEMM, |y|^2, y->SBUF -----------
            yg = yp.tile([P, TG, D + 1], bf16, tag="yg", name="yg")
            ny2 = sct("ny2")
            for t in range(TG):
                xn = xnp.tile([P, D], bf16, tag="xn", name="xn")
                V.tensor_scalar(xn[:], hg[:, t, :], mu[:, t : t + 1], gsc[:, t : t + 1],
                                 OP.subtract, OP.mult)
                ps = pst.tile([P, D], bf16, tag="ps", name="ps")
                PE.transpose(ps[:, 0:P], xn[:, 0:P], ident[:])
                PE.transpose(ps[:, P:D], xn[:, P:D], ident[:])
                xt = xtp.tile([P, D], bf16, tag="xt", name="xt")
                S.copy(xt[:], ps[:])
                yps = psy.tile([P, D + 1], f32, tag="yps", name="yps")
                PE.matmul(yps[:], xt[:, 0:P], rhs_sb[:, 0, :], start=True, stop=False)
                PE.matmul(yps[:], xt[:, P:D], rhs_sb[:, 1, :], start=False, stop=True)
                sq = scrp.tile([P, D], bf16, tag="sq", name="sq")
                S.activation(sq[:], yps[:, 0:D], FT.Square, accum_out=ny2[:, t : t + 1])
                V.tensor_copy(yg[:, t, :], yps[:])

            # --- phase B scalars (DVE + ACT): q' = q/pa, c4 = f_o*pa -----
            yv = sct("yv");    V.tensor_copy(yv[:], yg[:, :, D : D + 1])
            ny2c = sct("ny2c"); V.tensor_scalar_max(ny2c[:], ny2[:], TINY)
            lny = sct("lny");  S.activation(lny[:], ny2c[:], FT.Ln)
            ny = sct("ny");    S.activation(ny[:], lny[:], FT.Exp, scale=0.5)
            rny = sct("rny");  S.activation(rny[:], lny[:], FT.Exp, scale=-0.5)
            nt = sct("nt");    V.tensor_mul(nt[:], ny[:], s_[:])
            e2t = sct("e2t");  S.activation(e2t[:], nt[:], FT.Exp, scale=2.0)
            ue = sct("ue");    V.tensor_scalar_add(ue[:], e2t[:], 1.0)
            lue = sct("lue");  S.activation(lue[:], ue[:], FT.Ln)
            inv = sct("inv");  S.activation(inv[:], lue[:], FT.Exp, scale=-1.0)
            T_ = sct("T_");    V.tensor_scalar(T_[:], inv[:], -2.0, 1.0, OP.mult, OP.add)
            a = sct("a");      V.tensor_mul(a[:], T_[:], rny[:])
            res2 = sct("res2"); V.tensor_mul(res2[:], T_[:], T_[:])
            xv = sct("xv");    V.tensor_mul(xv[:], a[:], yv[:])
            tA = sct("tA");    V.tensor_scalar(tA[:], xv[:], 2.0, 1.0, OP.mult, OP.add)
            den = sct("den");  V.scalar_tensor_tensor(den[:], res2[:], v2, tA[:], OP.mult, OP.add)
            denc = sct("denc"); V.tensor_scalar_max(denc[:], den[:], TINY)
            lden = sct("lden"); S.activation(lden[:], denc[:], FT.Ln)
            rden = sct("rden"); S.activation(rden[:], lden[:], FT.Exp, scale=-1.0)
            tB = sct("tB");    V.tensor_scalar(tB[:], xv[:], 2.0, 1.0 + v2, OP.mult, OP.add)
            p_ = sct("p_");    V.tensor_mul(p_[:], tB[:], rden[:])
            tC = sct("tC");    V.tensor_scalar(tC[:], res2[:], -1.0, 1.0, OP.mult, OP.add)
            q_ = sct("q_");    V.tensor_mul(q_[:], tC[:], rden[:])
            pa = sct("pa");    V.tensor_mul(pa[:], p_[:], a[:])
            pac = sct("pac");  V.tensor_scalar_max(pac[:], pa[:], TINY)
            lpa = sct("lpa");  S.activation(lpa[:], pac[:], FT.Ln)
            rpa = sct("rpa");  S.activation(rpa[:], lpa[:], FT.Exp, scale=-1.0)
            qp = sct("qp");    V.tensor_mul(qp[:], q_[:], rpa[:])
            e1 = sct("e1");    V.tensor_mul(e1[:], pa[:], pa[:])
            e2_ = sct("e2_");  V.tensor_mul(e2_[:], e1[:], ny2c[:])
            e3 = sct("e3");    V.tensor_mul(e3[:], pa[:], q_[:])
            e4 = sct("e4");    V.scalar_tensor_tensor(e4[:], e3[:], 2.0, yv[:], OP.mult, OP.mult)
            e5 = sct("e5");    V.tensor_add(e5[:], e2_[:], e4[:])
            e6 = sct("e6");    V.tensor_mul(e6[:], q_[:], q_[:])
            no2 = sct("no2");  V.scalar_tensor_tensor(no2[:], e6[:], v2, e5[:], OP.mult, OP.add)
            no2c = sct("no2c"); V.tensor_scalar_max(no2c[:], no2[:], TINY)
            lno = sct("lno");  S.activation(lno[:], no2c[:], FT.Ln)
            no_ = sct("no_");  S.activation(no_[:], lno[:], FT.Exp, scale=0.5)
            rno = sct("rno");  S.activation(rno[:], lno[:], FT.Exp, scale=-0.5)
            z2 = sct("z2");    V.tensor_scalar_min(z2[:], no_[:], BOUND)
            n2 = sct("n2");    V.tensor_scalar_add(n2[:], z2[:], 1.0)
            dz = sct("dz");    V.tensor_scalar(dz[:], z2[:], -1.0, 1.0, OP.mult, OP.add)
            lA = sct("lA");    S.activation(lA[:], n2[:], FT.Ln)
            lB = sct("lB");    S.activation(lB[:], dz[:], FT.Ln)
            dlo = sct("dlo");  V.tensor_sub(dlo[:], lA[:], lB[:])
            c4a = sct("c4a");  V.scalar_tensor_tensor(c4a[:], dlo[:], 0.5, rno[:], OP.mult, OP.mult)
            c4 = sct("c4");    V.tensor_mul(c4[:], c4a[:], pa[:])

            # --- per-tile: t2' = q'*v + y ; nr2raw = sum(relu(t2')^2) ----
            t2g = t2p.tile([P, TG, D], bf16, tag="t2g", name="t2g")
            nr2 = sct("nr2")
            for t in range(TG):
                V.scalar_tensor_tensor(t2g[:, t, :], vb_sb[:], qp[:, t : t + 1],
                                       yg[:, t, 0:D], OP.mult, OP.add)
                s2 = scrp.tile([P, D], bf16, tag="s2", name="s2")
                V.scalar_tensor_tensor(s2[:], t2g[:, t, :], 0.0, t2g[:, t, :],
                                       OP.max, OP.mult, accum_out=nr2[:, t : t + 1])

            # --- phase C scalars: c34 = tanh(nr)/nr * f_o * pa -----------
            nr2c = sct("nr2c"); V.tensor_scalar_max(nr2c[:], nr2[:], TINY)
            h1 = sct("h1");    V.tensor_mul(h1[:], c4[:], c4[:])
            h2 = sct("h2");    V.tensor_mul(h2[:], h1[:], nr2c[:])
            lh2 = sct("lh2");  S.activation(lh2[:], h2[:], FT.Ln)
            nrr = sct("nrr");  S.activation(nrr[:], lh2[:], FT.Exp, scale=0.5)
            eee = sct("eee");  S.activation(eee[:], nrr[:], FT.Exp, scale=2.0)
            uu = sct("uu");    V.tensor_scalar_add(uu[:], eee[:], 1.0)
            luu = sct("luu");  S.activation(luu[:], uu[:], FT.Ln)
            innv = sct("innv"); S.activation(innv[:], luu[:], FT.Exp, scale=-1.0)
            Tn = sct("Tn");    V.tensor_scalar(Tn[:], innv[:], -2.0, 1.0, OP.mult, OP.add)
            lnr = sct("lnr");  S.activation(lnr[:], nr2c[:], FT.Ln)
            rrw = sct("rrw");  S.activation(rrw[:], lnr[:], FT.Exp, scale=-0.5)
            c34 = sct("c34");  V.tensor_mul(c34[:], Tn[:], rrw[:])

            # --- per-tile: out = max(c34 * t2', 0), store ----------------
            for tc0 in range(0, TG, OG_CH):
                og = ogp.tile([P, OG_CH, D], f32, tag="og", name="og")
                for ti in range(OG_CH):
                    t = tc0 + ti
                    V.tensor_scalar(og[:, ti, :], t2g[:, t, :], c34[:, t : t + 1],
                                     0.0, OP.mult, OP.max)
                oslab = out_ext[bass.ts(g * (TG // OG_CH) + tc0 // OG_CH, OG_CH * P), :] \
                    .rearrange("(t p) d -> p t d", p=P)
                nc.sync.dma_start(oslab, og[:])

    nc.compile()
    return nc


_NC_CACHE = {}


def _run_device(h, weight, bias):
    from concourse.bass_utils import run_bass_kernel_spmd
    import os

    # host-side prep of the tiny replicated params
    b = bias.reshape(-1).astype(np.float64)
    nb = max(float(np.linalg.norm(b)), 1e-15)
    v = (np.tanh(nb) / nb) * b                       # constant mobius 'second'
    v2 = float(np.tanh(nb) ** 2)                     # |v|^2
    wt = weight.astype(np.float64).T                 # [in, out] = W.T
    rhs = np.concatenate([wt, (wt @ v)[:, None]], axis=1)   # [256, 257]
    rhs_t = np.ascontiguousarray(
        rhs.reshape(2, P, D + 1).transpose(1, 0, 2)
    ).astype(_bf16)                                  # [128, 2, 257]
    vb = np.ascontiguousarray(
        np.broadcast_to(v.astype(_bf16), (P, D))
    )

    key = round(v2, 12)
    if key not in _NC_CACHE:
        _NC_CACHE[key] = _build_nc(v2)
    nc = _NC_CACHE[key]

    in_maps = []
    for c in range(N_CORES):
        in_maps.append({
            "h": np.ascontiguousarray(h[c * RPC : (c + 1) * RPC]),
            "rhs": rhs_t,
            "vb": vb,
        })

    trace = os.environ.get("HNN_TRACE", "0") == "1"
    tmpdir = os.environ.get("HNN_TRACE_DIR") or None
    try:
        res = run_bass_kernel_spmd(
            nc, in_maps, list(range(N_CORES)), trace=trace, tmpdir=tmpdir
        )
    except Exception:
        if not trace:
            raise
        res = run_bass_kernel_spmd(nc, in_maps, list(range(N_CORES)))
    global LAST_RESULTS
    LAST_RESULTS = res
    return np.concatenate([res.results[c]["out"] for c in range(N_CORES)], axis=0)


LAST_RESULTS = None


# ---------------------------------------------------------------------------
# numpy fallback (general gamma/beta or unexpected shapes)
def _np_norm(x):
    n = np.sqrt(np.sum(x * x, axis=-1, keepdims=True, dtype=np.float32))
    return np.maximum(n.astype(np.float32), np.float32(1e-15))


def _np_logmap0(x):
    n = _np_norm(x)
    z = np.minimum(n, np.float32(BOUND))
    return (np.arctanh(z) * x / n).astype(np.float32)


def _np_expmap0(u):
    n = _np_norm(u)
    return (np.tanh(n) * u / n).astype(np.float32)


def _np_kernel(h, weight, bias, gamma, beta):
    x = _np_logmap0(h)
    mu = np.mean(x, -1, keepdims=True, dtype=np.float32)
    var = np.mean((x - mu) ** 2, -1, keepdims=True, dtype=np.float32)
    x = ((x - mu) / np.sqrt(var + np.float32(LN_EPS)) * gamma + beta).astype(np.float32)
    hn = _np_expmap0(x)
    xt = (_np_logmap0(hn) @ weight.T).astype(np.float32)
    res = _np_expmap0(xt)
    r2 = np.sum(res * res, -1, keepdims=True, dtype=np.float32)
    b = (bias * (1.0 - r2)).astype(np.float32)
    n = _np_norm(b)
    lam = (2.0 / np.maximum(1.0 - r2, np.float32(1e-15))).astype(np.float32)
    second = (np.tanh(lam * n / 2.0) * b / n).astype(np.float32)
    x2 = np.sum(res * res, -1, keepdims=True, dtype=np.float32)
    y2 = np.sum(second * second, -1, keepdims=True, dtype=np.float32)
    xy = np.sum(res * second, -1, keepdims=True, dtype=np.float32)
    num = (1.0 + 2.0 * xy + y2).astype(np.float32) * res + (1.0 - x2).astype(np.float32) * second
    den = (1.0 + 2.0 * xy + x2 * y2).astype(np.float32)
    res = (num / np.maximum(den, np.float32(1e-15))).astype(np.float32)
    out = _np_expmap0(np.maximum(_np_logmap0(res), 0.0).astype(np.float32))
    return out.astype(np.float32)


def kernel(h, weight, bias, gamma, beta, **_unused):
    h = np.asarray(h, dtype=np.float32)
    weight = np.asarray(weight, dtype=np.float32)
    bias = np.asarray(bias, dtype=np.float32).reshape(1, D)
    gamma = np.asarray(gamma, dtype=np.float32)
    beta = np.asarray(beta, dtype=np.float32)

    std_shapes = (h.shape == (N_FULL, D) and weight.shape == (D, D))
    trivial_ln = bool(np.all(gamma == 1.0) and np.all(beta == 0.0))
    if std_shapes and trivial_ln:
        return _run_device(h, weight, bias)
    return _np_kernel(h, weight, bias, gamma, beta)

